# revision 6
# baseline (speedup 1.0000x reference)
"""Distributed Trainium2 Bass kernel for GQA attention (nn_Attention_27814208209106).

Sharding: 8 cores = 2 batches x 4 KV-head groups.
  Phase 1: per-core q/k/v projections (7 q-heads + 1 kv head) + RoPE.
  Phase 2: causal attention (k-stationary orientation, exp on ScalarE,
           softmax denominators via ones-matmul on PE).
  Phase 3: AllGather qkv^T (bf16) within each 4-core batch group, then
           o-proj over this core's 896-column slice of the output.
Host assembles out[b, :, 896*j:896*(j+1)] from core (b, j).

All matmuls in bf16 with f32 PSUM accumulation.
"""

import math
import numpy as np

import concourse.bass as bass
import concourse.mybir as mybir
import concourse.tile as tile
from concourse import bacc
from concourse.bass_utils import run_bass_kernel_spmd

P = 128
FB = 512  # psum free-dim block (f32 psum bank limit)
THETA = 1000000.0

F32 = mybir.dt.float32
BF16 = mybir.dt.bfloat16


class Cfg:
    def __init__(self, T=1024, EMB=3584, NH=28, KVH=4, HD=128):
        self.T, self.EMB, self.NH, self.KVH, self.HD = T, EMB, NH, KVH, HD
        self.GQ = NH // KVH          # q heads per kv head (7)
        self.HG = self.GQ * HD       # per-core q width (896)
        self.NHD = NH * HD           # full qkv width (3584)
        self.EO = EMB // 4           # o-proj output slice per core (896)
        self.KT = EMB // P           # contraction tiles (28)
        self.TT = T // P             # token tiles (8)
        self.NB = (T + FB - 1) // FB  # 512-blocks of T
        self.scale = HD ** -0.5


def _t_blocks(cfg):
    """[(t0, w)] 512-aligned blocks covering [0, T)."""
    return [(b * FB, min(cfg.T, (b + 1) * FB) - b * FB) for b in range(cfg.NB)]


def _chunks_for_si(cfg, si):
    """512-boundary-aligned chunks covering [si*128, T) (causal span)."""
    out = []
    t = si * P
    while t < cfg.T:
        nxt = min(cfg.T, (t // FB + 1) * FB)
        out.append((t, nxt - t))
        t = nxt
    return out


def build_kernel(cfg: Cfg):
    nc = bacc.Bacc(
        "TRN2",
        target_bir_lowering=False,
        debug=False,
        enable_asserts=False,
        num_devices=8,
    )

    xb = nc.dram_tensor("xb", [cfg.T, cfg.EMB], F32, kind="ExternalInput").ap()
    wq_s = nc.dram_tensor("wq_s", [cfg.EMB, cfg.HG], F32, kind="ExternalInput").ap()
    wk_s = nc.dram_tensor("wk_s", [cfg.EMB, cfg.HD], F32, kind="ExternalInput").ap()
    wv_s = nc.dram_tensor("wv_s", [cfg.EMB, cfg.HD], F32, kind="ExternalInput").ap()
    wo_s = nc.dram_tensor("wo_s", [cfg.NHD, cfg.EO], F32, kind="ExternalInput").ap()
    cosT = nc.dram_tensor("cosT", [cfg.HD // 2, cfg.T], F32, kind="ExternalInput").ap()
    sinT = nc.dram_tensor("sinT", [cfg.HD // 2, cfg.T], F32, kind="ExternalInput").ap()
    o_s = nc.dram_tensor("o_s", [cfg.T, cfg.EO], F32, kind="ExternalOutput").ap()

    with tile.TileContext(nc) as tc:
        _body(tc, cfg, xb, wq_s, wk_s, wv_s, wo_s, cosT, sinT, o_s)

    nc.compile()
    return nc


def _body(tc, cfg, xb, wq_s, wk_s, wv_s, wo_s, cosT, sinT, o_s):
    nc = tc.nc
    H2 = cfg.HD // 2
    tblocks = _t_blocks(cfg)

    with (
        tc.tile_pool(name="const", bufs=1) as constp,
        tc.tile_pool(name="qT", bufs=cfg.GQ) as qTp,
        tc.tile_pool(name="kT", bufs=1) as kTp,
        tc.tile_pool(name="vv", bufs=cfg.TT) as vp,
        tc.tile_pool(name="dram", bufs=1, space="DRAM") as dramp,
    ):
        # --- constants ---
        ident = constp.tile([P, P], BF16, name="ident")
        nc.gpsimd.memset(ident, 0.0)
        nc.gpsimd.affine_select(
            out=ident, in_=ident, compare_op=mybir.AluOpType.not_equal,
            fill=1.0, base=0, pattern=[[-1, P]], channel_multiplier=1,
        )
        # dmask[s, t] = 1 if s <= t else 0  (valid keys in diag tile)
        dmask = constp.tile([P, P], BF16, name="dmask")
        nc.gpsimd.memset(dmask, 1.0)
        nc.gpsimd.affine_select(
            out=dmask, in_=dmask, compare_op=mybir.AluOpType.is_ge,
            fill=0.0, base=0, pattern=[[1, P]], channel_multiplier=-1,
        )
        ones_bf = constp.tile([P, 1], BF16, name="ones_bf")
        nc.vector.memset(ones_bf, 1.0)

        qT = [qTp.tile([P, cfg.T], BF16, name=f"qT{h}", tag="qT") for h in range(cfg.GQ)]
        kT = kTp.tile([P, cfg.T], BF16, name="kT")
        vts = [vp.tile([P, cfg.HD], BF16, name=f"v{i}", tag="v") for i in range(cfg.TT)]
        cc_in = dramp.tile([cfg.HG, cfg.T], BF16, name="cc_in")
        cc_out = dramp.tile([4 * cfg.HG, cfg.T], BF16, name="cc_out")

        # ================= Phase 1: x^T + projections =================
        with (
            tc.tile_pool(name="rope_cs", bufs=1) as csp,
            tc.tile_pool(name="xf", bufs=3) as xfp,
            tc.tile_pool(name="xh", bufs=3) as xhp,
            tc.tile_pool(name="xT", bufs=cfg.KT) as xTp,
            tc.tile_pool(name="wf", bufs=2) as wfp,
            tc.tile_pool(name="wqh", bufs=cfg.KT) as wqhp,
            tc.tile_pool(name="wkvh", bufs=2 * cfg.KT) as wkvhp,
            tc.tile_pool(name="ptr", bufs=4, space="PSUM") as ptrp,
            tc.tile_pool(name="pproj", bufs=2, space="PSUM") as pprojp,
            tc.tile_pool(name="pv", bufs=2, space="PSUM") as pvp,
            tc.tile_pool(name="rtmp", bufs=4) as rtp,
        ):
            cos_sb = csp.tile([H2, cfg.T], F32, name="cos_sb")
            sin_sb = csp.tile([H2, cfg.T], F32, name="sin_sb")
            nc.sync.dma_start(cos_sb, cosT)
            nc.sync.dma_start(sin_sb, sinT)

            # x load + cast + PE transpose -> xT (bf16, [EMB-part, T])
            # staged in column chunks of XC to bound SBUF staging space
            XC = min(cfg.EMB, 7 * P)
            xTt = [xTp.tile([P, cfg.T], BF16, name=f"xT{k}", tag="xT") for k in range(cfg.KT)]
            for ti in range(cfg.TT):
                for e0 in range(0, cfg.EMB, XC):
                    xf = xfp.tile([P, XC], F32, name="xf")
                    nc.sync.dma_start(xf, xb[ti * P:(ti + 1) * P, e0:e0 + XC])
                    xh = xhp.tile([P, XC], BF16, name="xh")
                    nc.any.tensor_copy(xh, xf)
                    for kei in range(XC // P):
                        ke = e0 // P + kei
                        ptr = ptrp.tile([P, P], BF16, name="ptr")
                        nc.tensor.transpose(ptr, xh[:, kei * P:(kei + 1) * P], ident)
                        nc.any.tensor_copy(xTt[ke][:, ti * P:(ti + 1) * P], ptr)

            # weights load + cast
            wqh = []
            for ke in range(cfg.KT):
                wf = wfp.tile([P, cfg.HG], F32, name="wf", tag="wf")
                nc.sync.dma_start(wf, wq_s[ke * P:(ke + 1) * P, :])
                wh = wqhp.tile([P, cfg.HG], BF16, name=f"wqh{ke}", tag="wqh")
                nc.any.tensor_copy(wh, wf)
                wqh.append(wh)
            wkh, wvh = [], []
            for ke in range(cfg.KT):
                wfk = wfp.tile([P, cfg.HD], F32, name="wfk", tag="wfk")
                nc.sync.dma_start(wfk, wk_s[ke * P:(ke + 1) * P, :])
                whk = wkvhp.tile([P, cfg.HD], BF16, name=f"wkh{ke}", tag="wkvh")
                nc.any.tensor_copy(whk, wfk)
                wkh.append(whk)
                wfv = wfp.tile([P, cfg.HD], F32, name="wfv", tag="wfk")
                nc.sync.dma_start(wfv, wv_s[ke * P:(ke + 1) * P, :])
                whv = wkvhp.tile([P, cfg.HD], BF16, name=f"wvh{ke}", tag="wkvh")
                nc.any.tensor_copy(whv, wfv)
                wvh.append(whv)

            def rope_drain(psum, dst, t0, w):
                """dst[:, t0:t0+w] = rope(psum) ; psum [128, w] f32."""
                c = cos_sb[:, t0:t0 + w]
                s = sin_sb[:, t0:t0 + w]
                p1 = psum[0:H2, :]
                p2 = psum[H2:P, :]
                t1 = rtp.tile([H2, FB], F32, name="t1", tag="rt1")[:, :w]
                t2 = rtp.tile([H2, FB], F32, name="t2", tag="rt2")[:, :w]
                nc.vector.tensor_mul(t1, p1, c)
                nc.vector.tensor_mul(t2, p2, s)
                nc.vector.tensor_sub(dst[0:H2, t0:t0 + w], t1, t2)
                nc.vector.tensor_mul(t1, p2, c)
                nc.vector.tensor_mul(t2, p1, s)
                nc.vector.tensor_add(dst[H2:P, t0:t0 + w], t1, t2)

            # q projection (per head, per 512-block), rope fused in drain
            for h in range(cfg.GQ):
                for (t0, w) in tblocks:
                    ps = pprojp.tile([P, FB], F32, name="psq", tag="pproj")[:, :w]
                    for ke in range(cfg.KT):
                        nc.tensor.matmul(
                            out=ps,
                            lhsT=wqh[ke][:, h * P:(h + 1) * P],
                            rhs=xTt[ke][:, t0:t0 + w],
                            start=(ke == 0), stop=(ke == cfg.KT - 1),
                        )
                    rope_drain(ps, qT[h], t0, w)

            # k projection + rope
            for (t0, w) in tblocks:
                ps = pprojp.tile([P, FB], F32, name="psk", tag="pproj")[:, :w]
                for ke in range(cfg.KT):
                    nc.tensor.matmul(
                        out=ps, lhsT=wkh[ke], rhs=xTt[ke][:, t0:t0 + w],
                        start=(ke == 0), stop=(ke == cfg.KT - 1),
                    )
                rope_drain(ps, kT, t0, w)

            # v projection: v[ti] = [128 tok, HD] (token-major, no rope)
            for ti in range(cfg.TT):
                ps = pvp.tile([P, cfg.HD], F32, name="psv", tag="pv")
                for ke in range(cfg.KT):
                    nc.tensor.matmul(
                        out=ps, lhsT=xTt[ke][:, ti * P:(ti + 1) * P], rhs=wvh[ke],
                        start=(ke == 0), stop=(ke == cfg.KT - 1),
                    )
                nc.any.tensor_copy(vts[ti], ps)

        # ================= Phase 2: attention =================
        with (
            tc.tile_pool(name="pl", bufs=2, space="PSUM") as plp,
            tc.tile_pool(name="psums", bufs=cfg.NB, space="PSUM") as psumsp,
            tc.tile_pool(name="pav", bufs=2, space="PSUM") as pavp,
            tc.tile_pool(name="pt", bufs=2 * cfg.TT, space="SBUF") as ptp,
            tc.tile_pool(name="rec", bufs=2) as recp,
            tc.tile_pool(name="recb", bufs=2) as recbp,
            tc.tile_pool(name="qkvT", bufs=cfg.GQ) as qkvTp,
        ):
            qkvT = [
                qkvTp.tile([P, cfg.T], BF16, name=f"qkvT{h}", tag="qkvT")
                for h in range(cfg.GQ)
            ]
            for h in range(cfg.GQ):
                pts = []
                # logits^T (k-stationary) + exp
                for si in range(cfg.TT):
                    pt = ptp.tile([P, cfg.T], BF16, name=f"pt{si}", tag="pt")
                    pts.append(pt)
                    for (t0, w) in _chunks_for_si(cfg, si):
                        pl = plp.tile([P, FB], F32, name="pl", tag="pl")[:, :w]
                        nc.tensor.matmul(
                            out=pl,
                            lhsT=kT[:, si * P:(si + 1) * P],
                            rhs=qT[h][:, t0:t0 + w],
                            start=True, stop=True,
                        )
                        nc.scalar.activation(
                            pt[:, t0 - si * P:t0 - si * P + w], pl,
                            mybir.ActivationFunctionType.Exp,
                            scale=cfg.scale,
                        )
                    # mask invalid (s > t) entries of the diagonal tile
                    nc.vector.tensor_mul(pt[:, 0:P], pt[:, 0:P], dmask)

                # denominators: sums[t] = sum_s pt[s, t] via ones-matmul
                sum_ps = []
                for bi, (t0, w) in enumerate(tblocks):
                    sp = psumsp.tile([1, FB], F32, name=f"sums{bi}", tag="sums")[:, :w]
                    sum_ps.append(sp)
                    si_last = min(cfg.TT - 1, ((t0 + w - 1) // P))
                    for si in range(si_last + 1):
                        c0 = max(t0, si * P)
                        cw = t0 + w - c0
                        nc.tensor.matmul(
                            out=sp[:, c0 - t0:c0 - t0 + cw],
                            lhsT=ones_bf,
                            rhs=pts[si][:, c0 - si * P:c0 - si * P + cw],
                            start=(si == 0), stop=(si == si_last),
                        )
                rec = recp.tile([1, cfg.T], F32, name="rec")
                for bi, (t0, w) in enumerate(tblocks):
                    nc.vector.reciprocal(rec[:, t0:t0 + w], sum_ps[bi])
                recb = recbp.tile([P, cfg.T], F32, name="recb")
                nc.gpsimd.partition_broadcast(recb, rec)

                # attn @ V  (v stationary) + normalize
                for bi, (t0, w) in enumerate(tblocks):
                    pav = pavp.tile([P, FB], F32, name="pav", tag="pav")[:, :w]
                    si_last = min(cfg.TT - 1, ((t0 + w - 1) // P))
                    for si in range(si_last + 1):
                        c0 = max(t0, si * P)
                        cw = t0 + w - c0
                        nc.tensor.matmul(
                            out=pav[:, c0 - t0:c0 - t0 + cw],
                            lhsT=vts[si],
                            rhs=pts[si][:, c0 - si * P:c0 - si * P + cw],
                            start=(si == 0), stop=(si == si_last),
                        )
                    nc.vector.tensor_mul(
                        qkvT[h][:, t0:t0 + w], pav, recb[:, t0:t0 + w]
                    )

                nc.sync.dma_start(cc_in[h * P:(h + 1) * P, :], qkvT[h])

        # ================= Phase 3: AllGather + o-proj =================
        nc.gpsimd.collective_compute(
            "AllGather",
            mybir.AluOpType.bypass,
            replica_groups=[[0, 1, 2, 3], [4, 5, 6, 7]],
            ins=[cc_in.opt()],
            outs=[cc_out.opt()],
        )

        KO = 4 * cfg.GQ  # 28 contraction tiles of the o-proj
        eblocks = [(e * FB, min(cfg.EO, (e + 1) * FB) - e * FB)
                   for e in range((cfg.EO + FB - 1) // FB)]
        with (
            tc.tile_pool(name="qkh", bufs=KO) as qkhp,
            tc.tile_pool(name="wof", bufs=2) as wofp,
            tc.tile_pool(name="woh", bufs=KO) as wohp,
            tc.tile_pool(name="po", bufs=4, space="PSUM") as pop,
            tc.tile_pool(name="osb", bufs=3) as osbp,
        ):
            qkh = []
            for kt in range(KO):
                q = qkhp.tile([P, cfg.T], BF16, name=f"qkh{kt}", tag="qkh")
                nc.sync.dma_start(q, cc_out[kt * P:(kt + 1) * P, :])
                qkh.append(q)
            woh = []
            for kt in range(KO):
                wf = wofp.tile([P, cfg.EO], F32, name="wof", tag="wof")
                nc.sync.dma_start(wf, wo_s[kt * P:(kt + 1) * P, :])
                wh = wohp.tile([P, cfg.EO], BF16, name=f"woh{kt}", tag="woh")
                nc.any.tensor_copy(wh, wf)
                woh.append(wh)

            for ti in range(cfg.TT):
                osb = osbp.tile([P, cfg.EO], F32, name="osb", tag="osb")
                for (e0, ew) in eblocks:
                    po = pop.tile([P, FB], F32, name="po", tag="po")[:, :ew]
                    for kt in range(KO):
                        nc.tensor.matmul(
                            out=po,
                            lhsT=qkh[kt][:, ti * P:(ti + 1) * P],
                            rhs=woh[kt][:, e0:e0 + ew],
                            start=(kt == 0), stop=(kt == KO - 1),
                        )
                    nc.any.tensor_copy(osb[:, e0:e0 + ew], po)
                nc.sync.dma_start(o_s[ti * P:(ti + 1) * P, :], osb)


# ======================= host side =======================

_NC_CACHE = {}


def _get_nc(cfg_key=None):
    if cfg_key not in _NC_CACHE:
        _NC_CACHE[cfg_key] = build_kernel(Cfg())
    return _NC_CACHE[cfg_key]


def _rope_tables(segment_ids, cur_ind, T, HD):
    valid = (np.asarray(segment_ids) != 0)
    pos = np.cumsum(valid, axis=-1) - 1 + int(cur_ind)  # [B, T]
    frac = 2.0 * np.arange(HD // 2, dtype=np.float64) / HD
    timescale = THETA ** frac
    ang = pos[..., None].astype(np.float64) / timescale  # [B, T, HD/2]
    cosT = np.transpose(np.cos(ang), (0, 2, 1)).astype(np.float32)  # [B, HD/2, T]
    sinT = np.transpose(np.sin(ang), (0, 2, 1)).astype(np.float32)
    return cosT, sinT


def prepare_in_maps(inputs, cfg=None):
    cfg = cfg or Cfg()
    x = np.ascontiguousarray(np.asarray(inputs["x"], dtype=np.float32))
    wq = np.asarray(inputs["wq"], dtype=np.float32)
    wk = np.asarray(inputs["wk"], dtype=np.float32)
    wv = np.asarray(inputs["wv"], dtype=np.float32)
    wo = np.asarray(inputs["wo"], dtype=np.float32)
    seg = np.asarray(inputs["segment_ids"])
    cur = int(np.asarray(inputs["cur_ind"]))

    B, T, EMB = x.shape
    assert (B, T, EMB) == (2, cfg.T, cfg.EMB)
    HG = cfg.HG
    cosT, sinT = _rope_tables(seg, cur, T, cfg.HD)

    in_maps = []
    for c in range(8):
        b, j = c // 4, c % 4
        in_maps.append({
            "xb": x[b],
            "wq_s": np.ascontiguousarray(wq[:, j * HG:(j + 1) * HG]),
            "wk_s": np.ascontiguousarray(wk[:, j * cfg.HD:(j + 1) * cfg.HD]),
            "wv_s": np.ascontiguousarray(wv[:, j * cfg.HD:(j + 1) * cfg.HD]),
            "wo_s": np.ascontiguousarray(wo[:, j * cfg.EO:(j + 1) * cfg.EO]),
            "cosT": np.ascontiguousarray(cosT[b]),
            "sinT": np.ascontiguousarray(sinT[b]),
        })
    return in_maps


def assemble_out(results, cfg=None):
    cfg = cfg or Cfg()
    out = np.empty((2, cfg.T, cfg.EMB), np.float32)
    for c in range(8):
        b, j = c // 4, c % 4
        out[b, :, j * cfg.EO:(j + 1) * cfg.EO] = results[c]["o_s"]
    return out


def kernel(**inputs):
    cfg = Cfg()
    in_maps = prepare_in_maps(inputs, cfg)
    nc = _get_nc()
    res = run_bass_kernel_spmd(nc, in_maps, core_ids=list(range(8)))
    return assemble_out(res.results, cfg)
